# revision 23
# baseline (speedup 1.0000x reference)
"""Trainium2 Bass kernel for nn_ChebychevInput.

out[b,o,s] = sum_{i,p} (WEIGHT_MAGNITUDE*coef[o,i,p]) * cos(p*arccos(x[b,i,s]))

Device pipeline per core (s-shard of 16384, both batches):
  theta-stage (tiny, flat [96,1024] layout):
      a = arctan(x/sqrt(1-x^2)) = arcsin(x);  theta = pi/2 - a
      theta' = theta * 2^16/(2pi)   (cycles in 2^16 units)
      split theta' = hi + lo (bf16 pair, exact to ~2^-9 * 32 = 0.0625 units),
      round-trip through DRAM into a [3, 16384] layout (partitions 0..2).
  per (b, s-chunk) of 1024 samples:
      PE    th3[126, SC] (42 rows per i) = E.T @ [theta_hi; theta_lo] --
            two accumulating bf16 matmuls into PSUM (exact ones-selector E)
      DVE   (x7 k-tiles): Y32 = int32(th3_psum * p + 0.25*2^16)   [one pass]
      ACT   one Sin over the int16-bitcast low halfwords: T = sin(2pi*Y/2^16)
            = cos(p*theta)   -> fp16
      PE    out[o,s] accumulated over 7 k-tiles: lhsT = W[126,128] fp16
      Pool  PSUM -> SBUF fp32, DMA -> out
Row packing: k-tile kt row j: i = j//42, p = 42*kt + j%42  (k=126 rows/tile).
"""
import sys

sys.path.insert(0, "/opt/trn_rl_repo")

import numpy as np

BATCH = 2
INPUT_DIM = 3
N_SAMPLES = 131072
OUTPUT_DIM = 256
POLY_DEGREE = 256  # p = 0..256 -> 257 values
N_CORES = 8
S_SHARD = N_SAMPLES // N_CORES  # 16384
SC = 1024                       # sample chunk
NSC = S_SHARD // SC             # 16
NKT = 7                         # k-tiles of 126 rows (3i x 42p)
KT_ROWS = 126
WEIGHT_MAGNITUDE = float(np.sqrt(6.0 / (INPUT_DIM * (POLY_DEGREE + 1))))
TWO16 = 65536.0
FP8_PAIRS = ((1, 2), (4, 5))  # k-tile pairs computed in fp8 e4m3 DoubleRow
FP8_KTS = tuple(kt for pr in FP8_PAIRS for kt in pr)
F16_KTS = tuple(kt for kt in range(NKT) if kt not in FP8_KTS)

_compiled = {}


def _build(reps=1):
    import concourse.tile as tile
    from concourse import bacc, mybir

    F32 = mybir.dt.float32
    F16 = mybir.dt.float16
    BF16 = mybir.dt.bfloat16
    I32 = mybir.dt.int32
    I16 = mybir.dt.int16
    AF = mybir.ActivationFunctionType
    ALU = mybir.AluOpType

    F8 = mybir.dt.float8e4
    nc = bacc.Bacc("TRN2", target_bir_lowering=False, debug=False)
    x_d = nc.dram_tensor("x", [BATCH, INPUT_DIM, S_SHARD], F32, kind="ExternalInput")
    w_d = nc.dram_tensor("w", [KT_ROWS, NKT * OUTPUT_DIM], F16, kind="ExternalInput")
    pc_d = nc.dram_tensor("pc", [KT_ROWS, NKT], F32, kind="ExternalInput")
    e_d = nc.dram_tensor("e", [INPUT_DIM, KT_ROWS], BF16, kind="ExternalInput")
    # fp8 weights for k-tile pairs FP8_PAIRS: cols (P, q, t, o); q=0 -> e4m3(w),
    # q=1 -> e4m3(w - e4m3(w)) error compensation. Both fed to DoubleRow
    # matmuls so each pair of k-tiles costs 0.5 cyc/row.
    NP8 = len(FP8_PAIRS)
    w8_d = nc.dram_tensor("w8", [KT_ROWS, NP8 * 2 * 2 * OUTPUT_DIM], F8,
                          kind="ExternalInput")
    out_d = nc.dram_tensor("out", [BATCH, OUTPUT_DIM, S_SHARD], F32, kind="ExternalOutput")
    # DRAM scratch for the theta round-trip (layout change [96,1024]->[3,16384]);
    # double-buffered by rep parity so rep r+1's theta stage does not WAR-stall
    # on rep r's tail reads.
    thhi_ds = [nc.dram_tensor(f"thhi_s{i}", [96, 1024], BF16, kind="Internal")
               for i in range(2)]
    thlo_ds = [nc.dram_tensor(f"thlo_s{i}", [96, 1024], BF16, kind="Internal")
               for i in range(2)]

    with tile.TileContext(nc) as tc:
        with (
            tc.tile_pool(name="const", bufs=1) as constp,
            tc.tile_pool(name="theta", bufs=1) as thp,
            tc.tile_pool(name="thr", bufs=2) as thrp,
            tc.tile_pool(name="yint", bufs=2) as yp,
            tc.tile_pool(name="tmat", bufs=2) as tp,
            tc.tile_pool(name="outs", bufs=3) as op,
            tc.tile_pool(name="psth", bufs=2, space="PSUM") as pth,
            tc.tile_pool(name="psum", bufs=4, space="PSUM") as pp,
        ):
            w_t = constp.tile([KT_ROWS, NKT * OUTPUT_DIM], F16)
            nc.sync.dma_start(w_t[:], w_d[:])
            pc_t = constp.tile([KT_ROWS, NKT], F32)
            nc.sync.dma_start(pc_t[:], pc_d[:])
            e_t = constp.tile([INPUT_DIM, KT_ROWS], BF16)
            nc.sync.dma_start(e_t[:], e_d[:])
            w8_t = constp.tile([KT_ROWS, NP8 * 2 * 2 * OUTPUT_DIM], F8)
            nc.sync.dma_start(w8_t[:], w8_d[:])

            for _rep in range(reps):
              # ---- theta stage: flat [96, 1024]; row = 48*b + 16*i + u, u = s-chunk
              xt = thp.tile([96, 1024], F32)
              nc.sync.dma_start(xt[:], x_d[:].rearrange("b i (u c) -> (b i u) c", c=1024))
              sq = thp.tile([96, 1024], F32)
              nc.scalar.activation(sq[:], xt[:], AF.Square)
              r2 = thp.tile([96, 1024], F32)
              nc.scalar.activation(r2[:], sq[:], AF.Sqrt, bias=1.0, scale=-1.0)
              inv = thp.tile([96, 1024], F32)
              nc.vector.reciprocal(inv[:], r2[:])
              q = thp.tile([96, 1024], F32)
              nc.vector.tensor_mul(q[:], xt[:], inv[:])
              asn = thp.tile([96, 1024], F32)
              nc.scalar.activation(asn[:], q[:], AF.Arctan)
              # theta' = (pi/2 - a) * 2^16/(2pi) = 2^14 - a * (2^16/2pi)
              thf = thp.tile([96, 1024], F32)
              nc.scalar.activation(thf[:], asn[:], AF.Copy,
                                   bias=16384.0, scale=float(-TWO16 / (2 * np.pi)))
              # split theta' into bf16 hi+lo (exact reconstruction to 2^-9*32)
              thhi = thp.tile([96, 1024], BF16)
              nc.vector.tensor_copy(thhi[:], thf[:])
              thlo = thp.tile([96, 1024], BF16)
              nc.vector.tensor_tensor(thlo[:], thf[:], thhi[:], ALU.subtract)
              # round-trip via DRAM to land [3, 16384] on partitions 0..2
              thhi_d = thhi_ds[_rep % 2]
              thlo_d = thlo_ds[_rep % 2]
              nc.sync.dma_start(thhi_d[:], thhi[:])
              nc.sync.dma_start(thlo_d[:], thlo[:])
              thhi_v = thhi_d[:].rearrange("(b i u) c -> b i (u c)", b=2, i=3)
              thlo_v = thlo_d[:].rearrange("(b i u) c -> b i (u c)", b=2, i=3)

              # ---- main loops: groups of 8 chunks share one [3, 8*SC] theta slab
              for g in range(4):
                b = g // 2
                sc0 = (g % 2) * 8
                thr_hi = thrp.tile([INPUT_DIM, 8 * SC], BF16, tag="hi")
                nc.sync.dma_start(thr_hi[:], thhi_v[b, :, sc0 * SC:(sc0 + 8) * SC])
                thr_lo = thrp.tile([INPUT_DIM, 8 * SC], BF16, tag="lo")
                nc.sync.dma_start(thr_lo[:], thlo_v[b, :, sc0 * SC:(sc0 + 8) * SC])
                for u in range(8):
                    sc = sc0 + u
                    # PE broadcast: th3[j,:] = theta'_{j//42}  (fp32 PSUM accum;
                    # matmul output is limited to one PSUM bank = 512 fp32)
                    th3 = pth.tile([KT_ROWS, SC], F32)
                    for hb in range(SC // 512):
                        cs = slice(u * SC + hb * 512, u * SC + hb * 512 + 512)
                        nc.tensor.matmul(th3[:, hb * 512:(hb + 1) * 512],
                                         e_t[:], thr_hi[:, cs],
                                         start=True, stop=False)
                        nc.tensor.matmul(th3[:, hb * 512:(hb + 1) * 512],
                                         e_t[:], thr_lo[:, cs],
                                         start=False, stop=True)
                    # Drain th3 to SBUF once (PSUM reads cost 2x on DVE/ACT,
                    # and Pool cannot read PSUM at all).
                    th3s = thrp.tile([KT_ROWS, SC], F32, tag="th3s")
                    nc.vector.tensor_copy(th3s[:], th3[:])
                    y32 = yp.tile([KT_ROWS, NKT * SC], I32)
                    for kt in range(NKT):
                        # y32 conversions split across Pool (idle) and DVE
                        eng = nc.gpsimd if kt < 4 else nc.vector
                        eng.tensor_scalar(
                            y32[:, kt * SC:(kt + 1) * SC], th3s[:],
                            pc_t[:, kt:kt + 1], 0.25 * TWO16, ALU.mult, ALU.add,
                        )
                    tm = tp.tile([KT_ROWS, NKT * SC], F16)
                    tm8 = tp.tile([KT_ROWS, 2 * NP8 * SC], F8, tag="tm8")
                    yv = y32[:].bitcast(I16).rearrange("p (n two) -> p n two", two=2)[:, :, 0]
                    sinscale = float(2 * np.pi / TWO16)
                    # fp16 Sin per remaining k-tile; fp8 Sin for the
                    # DoubleRow tiles (same total columns either way).
                    for kt in F16_KTS:
                        nc.scalar.activation(tm[:, kt * SC:(kt + 1) * SC],
                                             yv[:, kt * SC:(kt + 1) * SC],
                                             AF.Sin, scale=sinscale)
                    for t, kt in enumerate(FP8_KTS):
                        nc.scalar.activation(tm8[:, t * SC:(t + 1) * SC],
                                             yv[:, kt * SC:(kt + 1) * SC],
                                             AF.Sin, scale=sinscale)

                    w8v = w8_t[:].rearrange("p (P q t o) -> p P q t o", P=NP8, q=2, t=2)
                    for m in range(2):
                        for half in range(2):
                            ps = pp.tile([128, 512], F32)
                            for ki, kt in enumerate(F16_KTS):
                                nc.tensor.matmul(
                                    ps[:],
                                    w_t[:, kt * OUTPUT_DIM + m * 128: kt * OUTPUT_DIM + m * 128 + 128],
                                    tm[:, kt * SC + half * 512: kt * SC + half * 512 + 512],
                                    start=(ki == 0), stop=False,
                                )
                            for P in range(NP8):
                                tm8v = tm8[:, 2 * P * SC:2 * (P + 1) * SC] \
                                    .rearrange("p (t c) -> p t c", t=2)
                                for q in range(2):
                                    nc.tensor.matmul(
                                        ps[:],
                                        w8v[:, P, q, :, m * 128:m * 128 + 128],
                                        tm8v[:, :, half * 512:half * 512 + 512],
                                        start=False,
                                        stop=(P == NP8 - 1 and q == 1),
                                        perf_mode=mybir.MatmulPerfMode.DoubleRow,
                                    )
                            ob = op.tile([128, 512], F32)
                            nc.vector.tensor_copy(ob[:], ps[:])
                            nc.sync.dma_start(
                                out_d[b, m * 128:(m + 1) * 128,
                                      sc * SC + half * 512: sc * SC + half * 512 + 512],
                                ob[:],
                            )
    nc.compile()
    return nc


def _host_prep(coefficients):
    w = (coefficients.astype(np.float64) * WEIGHT_MAGNITUDE).astype(np.float32)
    # w: (256, 3, 257) -> lhsT rows j (i=j//42, p=42*kt+j%42), cols kt*256+o
    wk = np.zeros((KT_ROWS, NKT * OUTPUT_DIM), np.float32)
    j = np.arange(KT_ROWS)
    ii = j // 42
    for kt in range(NKT):
        pp_ = 42 * kt + (j % 42)
        valid = pp_ <= POLY_DEGREE
        # wk[j, kt*256 + o] = w[o, ii[j], pp_[j]]
        wk[valid, kt * OUTPUT_DIM:(kt + 1) * OUTPUT_DIM] = \
            w[:, ii[valid], pp_[valid]].T
    pc = np.zeros((KT_ROWS, NKT), np.float32)
    for kt in range(NKT):
        pc[:, kt] = 42 * kt + (j % 42)
    # ones-selector E for the PE broadcast: e[i, j] = (j // 42 == i)
    import ml_dtypes
    e = (ii[None, :] == np.arange(INPUT_DIM)[:, None]).astype(ml_dtypes.bfloat16)
    # fp8 DoubleRow weights for FP8_PAIRS with error compensation:
    # w8[:, P*1024 + q*512 + t*256 + o]: q=0 -> e4m3(w), q=1 -> e4m3(w - e4m3(w))
    w8 = np.zeros((KT_ROWS, len(FP8_PAIRS) * 2 * 2 * OUTPUT_DIM),
                  ml_dtypes.float8_e4m3)
    for P, pair in enumerate(FP8_PAIRS):
        for t, kt in enumerate(pair):
            wt = wk[:, kt * OUTPUT_DIM:(kt + 1) * OUTPUT_DIM]
            a = wt.astype(ml_dtypes.float8_e4m3)
            b = (wt - a.astype(np.float32)).astype(ml_dtypes.float8_e4m3)
            base = P * 1024
            w8[:, base + t * OUTPUT_DIM:base + (t + 1) * OUTPUT_DIM] = a
            w8[:, base + 512 + t * OUTPUT_DIM:base + 512 + (t + 1) * OUTPUT_DIM] = b
    return wk.astype(np.float16), pc, e, w8


def _get_callable(n_execs=1):
    """Build (once) a jitted shard_map callable running the bass program on 8 cores.

    n_execs>1 compiles a DIFFERENT bass program with the whole device pipeline
    repeated n_execs times (the neuronx_cc_hook only supports one bass_exec
    custom-call per jit, so chaining execs is not possible; on-device reps give
    the same differential-timing semantics with dispatch overhead cancelling).

    Inputs (globals, concat on axis 0 across cores):
      xg [8*2, 3, S_SHARD] f32, wg [8*126, 1792] f16, pcg [8*126, 7] f32,
      eg [8*3, 126] bf16.
    Returns out global [8*2, 256, S_SHARD] f32.
    """
    key = ("fn", n_execs)
    if key in _compiled:
        return _compiled[key]
    import jax
    import jax.numpy as jnp
    from jax.sharding import Mesh, PartitionSpec
    from jax.experimental.shard_map import shard_map
    from concourse import bass2jax
    from concourse.bass2jax import (
        _bass_exec_p, install_neuronx_cc_hook, partition_id_tensor)

    nckey = "nc" if n_execs == 1 else ("nc", n_execs)
    if nckey not in _compiled:
        _compiled[nckey] = _build(reps=n_execs)
    nc = _compiled[nckey]
    install_neuronx_cc_hook()

    pname = nc.partition_id_tensor.name if nc.partition_id_tensor else None
    in_names = ("x", "w", "pc", "e", "w8", "out") + ((pname,) if pname else ())
    out_names = ("out",)
    out_aval = jax.core.ShapedArray((BATCH, OUTPUT_DIM, S_SHARD), np.float32)

    def _body(xs, ws, pcs, es, w8s, zs):
        operands = [xs, ws, pcs, es, w8s, zs]
        if pname:
            operands.append(partition_id_tensor())
        outs = _bass_exec_p.bind(
            *operands,
            out_avals=(out_aval,),
            in_names=in_names,
            out_names=out_names,
            lowering_input_output_aliases=(),
            sim_require_finite=True,
            sim_require_nnan=True,
            nc=nc,
        )
        return outs[0]

    devices = jax.devices()[:N_CORES]
    mesh = Mesh(np.asarray(devices), ("core",))
    fn = jax.jit(shard_map(
        _body, mesh=mesh,
        in_specs=(PartitionSpec("core"),) * 6,
        out_specs=PartitionSpec("core"),
        check_rep=False,
    ))
    _compiled[key] = fn
    return fn


def _make_zeros():
    """Fresh on-device zero output buffers (donated into each kernel call)."""
    import jax
    import jax.numpy as jnp
    from jax.sharding import Mesh, PartitionSpec, NamedSharding

    if "zmk" not in _compiled:
        devices = jax.devices()[:N_CORES]
        mesh = Mesh(np.asarray(devices), ("core",))
        sh = NamedSharding(mesh, PartitionSpec("core"))
        _compiled["zmk"] = jax.jit(
            lambda: jnp.zeros((N_CORES * BATCH, OUTPUT_DIM, S_SHARD), np.float32),
            out_shardings=sh)
    return _compiled["zmk"]()


def _prep_globals(x, coefficients):
    wk, pc, e, w8 = _host_prep(coefficients)
    xg = np.ascontiguousarray(
        np.asarray(x, dtype=np.float32).reshape(BATCH, INPUT_DIM, N_CORES, S_SHARD)
        .transpose(2, 0, 1, 3).reshape(N_CORES * BATCH, INPUT_DIM, S_SHARD))
    wg = np.tile(wk, (N_CORES, 1))
    pcg = np.tile(pc, (N_CORES, 1))
    eg = np.tile(e, (N_CORES, 1))
    w8g = np.tile(w8, (N_CORES, 1))
    return xg, wg, pcg, eg, w8g


def kernel(x, coefficients):
    from concourse import bass2jax

    if "nc" not in _compiled:
        _compiled["nc"] = _build()
    nc = _compiled["nc"]
    wk, pc, e, w8 = _host_prep(coefficients)
    x = np.asarray(x, dtype=np.float32)
    in_maps = [
        {"x": np.ascontiguousarray(x[:, :, c * S_SHARD:(c + 1) * S_SHARD]),
         "w": wk, "pc": pc, "e": e, "w8": w8}
        for c in range(N_CORES)
    ]
    results = bass2jax.run_bass_via_pjrt(nc, in_maps, n_cores=N_CORES)
    out = np.concatenate([results[c]["out"] for c in range(N_CORES)], axis=2)
    return np.ascontiguousarray(out.astype(np.float32))


# revision 29
# speedup vs baseline: 1.1725x; 1.1725x over previous
"""Trainium2 Bass kernel for nn_ChebychevInput.

out[b,o,s] = sum_{i,p} (WEIGHT_MAGNITUDE*coef[o,i,p]) * cos(p*arccos(x[b,i,s]))

Device pipeline per core (s-shard of 16384, both batches):
  theta-stage (tiny, flat [96,1024] layout):
      a = arctan(x/sqrt(1-x^2)) = arcsin(x);  theta = pi/2 - a
      theta' = theta * 2^16/(2pi)   (cycles in 2^16 units)
      split theta' = hi + lo (bf16 pair, exact to ~2^-9 * 32 = 0.0625 units),
      round-trip through DRAM into a [3, 16384] layout (partitions 0..2).
  per (b, s-chunk) of 1024 samples:
      PE    th3[126, SC] (42 rows per i) = E.T @ [theta_hi; theta_lo] --
            two accumulating bf16 matmuls into PSUM (exact ones-selector E)
      DVE   (x7 k-tiles): Y32 = int32(th3_psum * p + 0.25*2^16)   [one pass]
      ACT   one Sin over the int16-bitcast low halfwords: T = sin(2pi*Y/2^16)
            = cos(p*theta)   -> fp16
      PE    out[o,s] accumulated over 7 k-tiles: lhsT = W[126,128] fp16
      Pool  PSUM -> SBUF fp32, DMA -> out
Row packing: k-tile kt row j: i = j//42, p = 42*kt + j%42  (k=126 rows/tile).
"""
import sys

sys.path.insert(0, "/opt/trn_rl_repo")

import numpy as np

BATCH = 2
INPUT_DIM = 3
N_SAMPLES = 131072
OUTPUT_DIM = 256
POLY_DEGREE = 256  # p = 0..256 -> 257 values
N_CORES = 8
S_SHARD = N_SAMPLES // N_CORES  # 16384
SC = 1024                       # sample chunk
NSC = S_SHARD // SC             # 16
NKT = 7                         # k-tiles of 126 rows (3i x 42p)
KT_ROWS = 126
WEIGHT_MAGNITUDE = float(np.sqrt(6.0 / (INPUT_DIM * (POLY_DEGREE + 1))))
TWO16 = 65536.0
FP8_PAIRS = ((1, 2), (4, 5))  # k-tile pairs computed in fp8 e4m3 DoubleRow
FP8_KTS = tuple(kt for pr in FP8_PAIRS for kt in pr)
F16_KTS = tuple(kt for kt in range(NKT) if kt not in FP8_KTS)
# y32/tm column slot per k-tile: fp16 tiles first (slots 0..2, one fp16 Sin
# over a contiguous range), fp8 tiles after (slots 3..6, one fp8 Sin).
SLOT = {kt: s for s, kt in enumerate(F16_KTS + FP8_KTS)}

_compiled = {}


def _build(reps=1):
    import concourse.tile as tile
    from concourse import bacc, mybir

    F32 = mybir.dt.float32
    F16 = mybir.dt.float16
    BF16 = mybir.dt.bfloat16
    I32 = mybir.dt.int32
    I16 = mybir.dt.int16
    AF = mybir.ActivationFunctionType
    ALU = mybir.AluOpType

    F8 = mybir.dt.float8e4
    nc = bacc.Bacc("TRN2", target_bir_lowering=False, debug=False)
    x_d = nc.dram_tensor("x", [BATCH, INPUT_DIM, S_SHARD], F32, kind="ExternalInput")
    w_d = nc.dram_tensor("w", [KT_ROWS, NKT * OUTPUT_DIM], F16, kind="ExternalInput")
    pc_d = nc.dram_tensor("pc", [KT_ROWS, NKT], F32, kind="ExternalInput")
    e_d = nc.dram_tensor("e", [INPUT_DIM, KT_ROWS], BF16, kind="ExternalInput")
    # fp8 weights for k-tile pairs FP8_PAIRS: cols (P, q, t, o); q=0 -> e4m3(w),
    # q=1 -> e4m3(w - e4m3(w)) error compensation. Both fed to DoubleRow
    # matmuls so each pair of k-tiles costs 0.5 cyc/row.
    NP8 = len(FP8_PAIRS)
    w8_d = nc.dram_tensor("w8", [KT_ROWS, NP8 * 2 * 2 * OUTPUT_DIM], F8,
                          kind="ExternalInput")
    out_d = nc.dram_tensor("out", [BATCH, OUTPUT_DIM, S_SHARD], F32, kind="ExternalOutput")
    # DRAM scratch for the theta round-trip (layout change [96,1024]->[3,16384]);
    # double-buffered by rep parity so rep r+1's theta stage does not WAR-stall
    # on rep r's tail reads.
    thhi_ds = [nc.dram_tensor(f"thhi_s{i}", [96, 1024], BF16, kind="Internal")
               for i in range(2)]
    thlo_ds = [nc.dram_tensor(f"thlo_s{i}", [96, 1024], BF16, kind="Internal")
               for i in range(2)]

    with tile.TileContext(nc) as tc:
        with (
            tc.tile_pool(name="const", bufs=1) as constp,
            tc.tile_pool(name="theta", bufs=1) as thp,
            tc.tile_pool(name="thr", bufs=2) as thrp,
            tc.tile_pool(name="yint", bufs=2) as yp,
            tc.tile_pool(name="tmat", bufs=3) as tp,
            tc.tile_pool(name="outs", bufs=4) as op,
            tc.tile_pool(name="psth", bufs=2, space="PSUM") as pth,
            tc.tile_pool(name="psum", bufs=4, space="PSUM") as pp,
        ):
            w_t = constp.tile([KT_ROWS, NKT * OUTPUT_DIM], F16)
            nc.sync.dma_start(w_t[:], w_d[:])
            pc_t = constp.tile([KT_ROWS, NKT], F32)
            nc.sync.dma_start(pc_t[:], pc_d[:])
            e_t = constp.tile([INPUT_DIM, KT_ROWS], BF16)
            nc.sync.dma_start(e_t[:], e_d[:])
            w8_t = constp.tile([KT_ROWS, NP8 * 2 * 2 * OUTPUT_DIM], F8)
            nc.sync.dma_start(w8_t[:], w8_d[:])

            for _rep in range(reps):
              # ---- theta stage: flat [96, 1024]; row = 48*b + 16*i + u, u = s-chunk
              xt = thp.tile([96, 1024], F32)
              nc.sync.dma_start(xt[:], x_d[:].rearrange("b i (u c) -> (b i u) c", c=1024))
              sq = thp.tile([96, 1024], F32)
              nc.scalar.activation(sq[:], xt[:], AF.Square)
              r2 = thp.tile([96, 1024], F32)
              nc.scalar.activation(r2[:], sq[:], AF.Sqrt, bias=1.0, scale=-1.0)
              inv = thp.tile([96, 1024], F32)
              nc.vector.reciprocal(inv[:], r2[:])
              q = thp.tile([96, 1024], F32)
              nc.vector.tensor_mul(q[:], xt[:], inv[:])
              asn = thp.tile([96, 1024], F32)
              nc.scalar.activation(asn[:], q[:], AF.Arctan)
              # theta' = (pi/2 - a) * 2^16/(2pi) = 2^14 - a * (2^16/2pi)
              thf = thp.tile([96, 1024], F32)
              nc.scalar.activation(thf[:], asn[:], AF.Copy,
                                   bias=16384.0, scale=float(-TWO16 / (2 * np.pi)))
              # split theta' into bf16 hi+lo (exact reconstruction to 2^-9*32)
              thhi = thp.tile([96, 1024], BF16)
              nc.vector.tensor_copy(thhi[:], thf[:])
              thlo = thp.tile([96, 1024], BF16)
              nc.vector.tensor_tensor(thlo[:], thf[:], thhi[:], ALU.subtract)
              # round-trip via DRAM to land [3, 16384] on partitions 0..2
              thhi_d = thhi_ds[_rep % 2]
              thlo_d = thlo_ds[_rep % 2]
              nc.sync.dma_start(thhi_d[:], thhi[:])
              nc.sync.dma_start(thlo_d[:], thlo[:])
              thhi_v = thhi_d[:].rearrange("(b i u) c -> b i (u c)", b=2, i=3)
              thlo_v = thlo_d[:].rearrange("(b i u) c -> b i (u c)", b=2, i=3)

              # ---- main loops: groups of 4 chunks share one [3, 4*SC] theta slab
              for g in range(8):
                b = g // 4
                sc0 = (g % 4) * 4
                thr_hi = thrp.tile([INPUT_DIM, 4 * SC], BF16, tag="hi")
                nc.sync.dma_start(thr_hi[:], thhi_v[b, :, sc0 * SC:(sc0 + 4) * SC])
                thr_lo = thrp.tile([INPUT_DIM, 4 * SC], BF16, tag="lo")
                nc.sync.dma_start(thr_lo[:], thlo_v[b, :, sc0 * SC:(sc0 + 4) * SC])
                for u in range(4):
                    sc = sc0 + u
                    # PE broadcast: th3[j,:] = theta'_{j//42}  (fp32 PSUM accum;
                    # matmul output is limited to one PSUM bank = 512 fp32)
                    th3 = pth.tile([KT_ROWS, SC], F32)
                    for hb in range(SC // 512):
                        cs = slice(u * SC + hb * 512, u * SC + hb * 512 + 512)
                        nc.tensor.matmul(th3[:, hb * 512:(hb + 1) * 512],
                                         e_t[:], thr_hi[:, cs],
                                         start=True, stop=False)
                        nc.tensor.matmul(th3[:, hb * 512:(hb + 1) * 512],
                                         e_t[:], thr_lo[:, cs],
                                         start=False, stop=True)
                    # Drain th3 to SBUF once (PSUM reads cost 2x on DVE/ACT,
                    # and Pool cannot read PSUM at all).
                    th3s = thrp.tile([KT_ROWS, SC], F32, tag="th3s")
                    nc.vector.tensor_copy(th3s[:], th3[:])
                    y32 = yp.tile([KT_ROWS, NKT * SC], I32)
                    for kt in range(NKT):
                        # y32 conversions split across Pool (idle) and DVE;
                        # written at the k-tile's column SLOT
                        eng = nc.gpsimd if kt < 4 else nc.vector
                        s = SLOT[kt]
                        eng.tensor_scalar(
                            y32[:, s * SC:(s + 1) * SC], th3s[:],
                            pc_t[:, kt:kt + 1], 0.25 * TWO16, ALU.mult, ALU.add,
                        )
                    NF16 = len(F16_KTS)
                    tm = tp.tile([KT_ROWS, NF16 * SC], F16)
                    tm8 = tp.tile([KT_ROWS, 2 * NP8 * SC], F8, tag="tm8")
                    yv = y32[:].bitcast(I16).rearrange("p (n two) -> p n two", two=2)[:, :, 0]
                    sinscale = float(2 * np.pi / TWO16)
                    # slots 0..NF16-1 are the fp16 tiles (one Sin), slots
                    # NF16.. are the fp8 DoubleRow tiles (one Sin).
                    nc.scalar.activation(tm[:], yv[:, 0:NF16 * SC],
                                         AF.Sin, scale=sinscale)
                    nc.scalar.activation(tm8[:], yv[:, NF16 * SC:NKT * SC],
                                         AF.Sin, scale=sinscale)

                    w8v = w8_t[:].rearrange("p (P q t o) -> p P q t o", P=NP8, q=2, t=2)
                    for m in range(2):
                        for half in range(2):
                            ps = pp.tile([128, 512], F32)
                            for ki in range(NF16):
                                kt = F16_KTS[ki]
                                nc.tensor.matmul(
                                    ps[:],
                                    w_t[:, kt * OUTPUT_DIM + m * 128: kt * OUTPUT_DIM + m * 128 + 128],
                                    tm[:, ki * SC + half * 512: ki * SC + half * 512 + 512],
                                    start=(ki == 0), stop=False,
                                )
                            for P in range(NP8):
                                tm8v = tm8[:, 2 * P * SC:2 * (P + 1) * SC] \
                                    .rearrange("p (t c) -> p t c", t=2)
                                for q in range(2):
                                    nc.tensor.matmul(
                                        ps[:],
                                        w8v[:, P, q, :, m * 128:m * 128 + 128],
                                        tm8v[:, :, half * 512:half * 512 + 512],
                                        start=False,
                                        stop=(P == NP8 - 1 and q == 1),
                                        perf_mode=mybir.MatmulPerfMode.DoubleRow,
                                    )
                            ob = op.tile([128, 512], F32)
                            nc.vector.tensor_copy(ob[:], ps[:])
                            nc.sync.dma_start(
                                out_d[b, m * 128:(m + 1) * 128,
                                      sc * SC + half * 512: sc * SC + half * 512 + 512],
                                ob[:],
                            )
    nc.compile()
    return nc


def _host_prep(coefficients):
    w = (coefficients.astype(np.float64) * WEIGHT_MAGNITUDE).astype(np.float32)
    # w: (256, 3, 257) -> lhsT rows j (i=j//42, p=42*kt+j%42), cols kt*256+o
    wk = np.zeros((KT_ROWS, NKT * OUTPUT_DIM), np.float32)
    j = np.arange(KT_ROWS)
    ii = j // 42
    for kt in range(NKT):
        pp_ = 42 * kt + (j % 42)
        valid = pp_ <= POLY_DEGREE
        # wk[j, kt*256 + o] = w[o, ii[j], pp_[j]]
        wk[valid, kt * OUTPUT_DIM:(kt + 1) * OUTPUT_DIM] = \
            w[:, ii[valid], pp_[valid]].T
    pc = np.zeros((KT_ROWS, NKT), np.float32)
    for kt in range(NKT):
        pc[:, kt] = 42 * kt + (j % 42)
    # ones-selector E for the PE broadcast: e[i, j] = (j // 42 == i)
    import ml_dtypes
    e = (ii[None, :] == np.arange(INPUT_DIM)[:, None]).astype(ml_dtypes.bfloat16)
    # fp8 DoubleRow weights for FP8_PAIRS with error compensation:
    # w8[:, P*1024 + q*512 + t*256 + o]: q=0 -> e4m3(w), q=1 -> e4m3(w - e4m3(w))
    w8 = np.zeros((KT_ROWS, len(FP8_PAIRS) * 2 * 2 * OUTPUT_DIM),
                  ml_dtypes.float8_e4m3)
    for P, pair in enumerate(FP8_PAIRS):
        for t, kt in enumerate(pair):
            wt = wk[:, kt * OUTPUT_DIM:(kt + 1) * OUTPUT_DIM]
            a = wt.astype(ml_dtypes.float8_e4m3)
            b = (wt - a.astype(np.float32)).astype(ml_dtypes.float8_e4m3)
            base = P * 1024
            w8[:, base + t * OUTPUT_DIM:base + (t + 1) * OUTPUT_DIM] = a
            w8[:, base + 512 + t * OUTPUT_DIM:base + 512 + (t + 1) * OUTPUT_DIM] = b
    return wk.astype(np.float16), pc, e, w8


def _get_callable(n_execs=1):
    """Build (once) a jitted shard_map callable running the bass program on 8 cores.

    n_execs>1 compiles a DIFFERENT bass program with the whole device pipeline
    repeated n_execs times (the neuronx_cc_hook only supports one bass_exec
    custom-call per jit, so chaining execs is not possible; on-device reps give
    the same differential-timing semantics with dispatch overhead cancelling).

    Inputs (globals, concat on axis 0 across cores):
      xg [8*2, 3, S_SHARD] f32, wg [8*126, 1792] f16, pcg [8*126, 7] f32,
      eg [8*3, 126] bf16.
    Returns out global [8*2, 256, S_SHARD] f32.
    """
    key = ("fn", n_execs)
    if key in _compiled:
        return _compiled[key]
    import jax
    import jax.numpy as jnp
    from jax.sharding import Mesh, PartitionSpec
    from jax.experimental.shard_map import shard_map
    from concourse import bass2jax
    from concourse.bass2jax import (
        _bass_exec_p, install_neuronx_cc_hook, partition_id_tensor)

    nckey = "nc" if n_execs == 1 else ("nc", n_execs)
    if nckey not in _compiled:
        _compiled[nckey] = _build(reps=n_execs)
    nc = _compiled[nckey]
    install_neuronx_cc_hook()

    pname = nc.partition_id_tensor.name if nc.partition_id_tensor else None
    in_names = ("x", "w", "pc", "e", "w8", "out") + ((pname,) if pname else ())
    out_names = ("out",)
    out_aval = jax.core.ShapedArray((BATCH, OUTPUT_DIM, S_SHARD), np.float32)

    def _body(xs, ws, pcs, es, w8s, zs):
        operands = [xs, ws, pcs, es, w8s, zs]
        if pname:
            operands.append(partition_id_tensor())
        outs = _bass_exec_p.bind(
            *operands,
            out_avals=(out_aval,),
            in_names=in_names,
            out_names=out_names,
            lowering_input_output_aliases=(),
            sim_require_finite=True,
            sim_require_nnan=True,
            nc=nc,
        )
        return outs[0]

    devices = jax.devices()[:N_CORES]
    mesh = Mesh(np.asarray(devices), ("core",))
    fn = jax.jit(shard_map(
        _body, mesh=mesh,
        in_specs=(PartitionSpec("core"),) * 6,
        out_specs=PartitionSpec("core"),
        check_rep=False,
    ))
    _compiled[key] = fn
    return fn


def _make_zeros():
    """Fresh on-device zero output buffers (donated into each kernel call)."""
    import jax
    import jax.numpy as jnp
    from jax.sharding import Mesh, PartitionSpec, NamedSharding

    if "zmk" not in _compiled:
        devices = jax.devices()[:N_CORES]
        mesh = Mesh(np.asarray(devices), ("core",))
        sh = NamedSharding(mesh, PartitionSpec("core"))
        _compiled["zmk"] = jax.jit(
            lambda: jnp.zeros((N_CORES * BATCH, OUTPUT_DIM, S_SHARD), np.float32),
            out_shardings=sh)
    return _compiled["zmk"]()


def _prep_globals(x, coefficients):
    wk, pc, e, w8 = _host_prep(coefficients)
    xg = np.ascontiguousarray(
        np.asarray(x, dtype=np.float32).reshape(BATCH, INPUT_DIM, N_CORES, S_SHARD)
        .transpose(2, 0, 1, 3).reshape(N_CORES * BATCH, INPUT_DIM, S_SHARD))
    wg = np.tile(wk, (N_CORES, 1))
    pcg = np.tile(pc, (N_CORES, 1))
    eg = np.tile(e, (N_CORES, 1))
    w8g = np.tile(w8, (N_CORES, 1))
    return xg, wg, pcg, eg, w8g


def kernel(x, coefficients):
    from concourse import bass2jax

    if "nc" not in _compiled:
        _compiled["nc"] = _build()
    nc = _compiled["nc"]
    wk, pc, e, w8 = _host_prep(coefficients)
    x = np.asarray(x, dtype=np.float32)
    in_maps = [
        {"x": np.ascontiguousarray(x[:, :, c * S_SHARD:(c + 1) * S_SHARD]),
         "w": wk, "pc": pc, "e": e, "w8": w8}
        for c in range(N_CORES)
    ]
    results = bass2jax.run_bass_via_pjrt(nc, in_maps, n_cores=N_CORES)
    out = np.concatenate([results[c]["out"] for c in range(N_CORES)], axis=2)
    return np.ascontiguousarray(out.astype(np.float32))


# revision 30
# speedup vs baseline: 1.3504x; 1.1517x over previous
"""Trainium2 Bass kernel for nn_ChebychevInput.

out[b,o,s] = sum_{i,p} (WEIGHT_MAGNITUDE*coef[o,i,p]) * cos(p*arccos(x[b,i,s]))

Device pipeline per core (s-shard of 16384, both batches):
  theta-stage (tiny, flat [96,1024] layout):
      a = arctan(x/sqrt(1-x^2)) = arcsin(x);  theta = pi/2 - a
      theta' = theta * 2^16/(2pi)   (cycles in 2^16 units)
      split theta' = hi + lo (bf16 pair, exact to ~2^-9 * 32 = 0.0625 units),
      round-trip through DRAM into a [3, 16384] layout (partitions 0..2).
  per (b, s-chunk) of 1024 samples:
      PE    th3[126, SC] (42 rows per i) = E.T @ [theta_hi; theta_lo] --
            two accumulating bf16 matmuls into PSUM (exact ones-selector E)
      DVE   (x7 k-tiles): Y32 = int32(th3_psum * p + 0.25*2^16)   [one pass]
      ACT   one Sin over the int16-bitcast low halfwords: T = sin(2pi*Y/2^16)
            = cos(p*theta)   -> fp16
      PE    out[o,s] accumulated over 7 k-tiles: lhsT = W[126,128] fp16
      Pool  PSUM -> SBUF fp32, DMA -> out
Row packing: k-tile kt row j: i = j//42, p = 42*kt + j%42  (k=126 rows/tile).
"""
import sys

sys.path.insert(0, "/opt/trn_rl_repo")

import numpy as np

BATCH = 2
INPUT_DIM = 3
N_SAMPLES = 131072
OUTPUT_DIM = 256
POLY_DEGREE = 256  # p = 0..256 -> 257 values
N_CORES = 8
S_SHARD = N_SAMPLES // N_CORES  # 16384
SC = 1024                       # sample chunk
NSC = S_SHARD // SC             # 16
NKT = 7                         # k-tiles of 126 rows (3i x 42p)
KT_ROWS = 126
WEIGHT_MAGNITUDE = float(np.sqrt(6.0 / (INPUT_DIM * (POLY_DEGREE + 1))))
TWO16 = 65536.0
FP8_KTS = (2, 4)  # k-tiles computed in fp8 e4m3 via DoubleRow matmuls

_compiled = {}


def _build(reps=1):
    import concourse.tile as tile
    from concourse import bacc, mybir

    F32 = mybir.dt.float32
    F16 = mybir.dt.float16
    BF16 = mybir.dt.bfloat16
    I32 = mybir.dt.int32
    I16 = mybir.dt.int16
    AF = mybir.ActivationFunctionType
    ALU = mybir.AluOpType

    F8 = mybir.dt.float8e4
    nc = bacc.Bacc("TRN2", target_bir_lowering=False, debug=False)
    x_d = nc.dram_tensor("x", [BATCH, INPUT_DIM, S_SHARD], F32, kind="ExternalInput")
    w_d = nc.dram_tensor("w", [KT_ROWS, NKT * OUTPUT_DIM], F16, kind="ExternalInput")
    pc_d = nc.dram_tensor("pc", [KT_ROWS, NKT], F32, kind="ExternalInput")
    e_d = nc.dram_tensor("e", [INPUT_DIM, KT_ROWS], BF16, kind="ExternalInput")
    # fp8 weights for k-tiles FP8_KTS: cols (q, t, o); q=0 -> e4m3(w),
    # q=1 -> e4m3(w - e4m3(w)) error compensation. Both fed to DoubleRow
    # matmuls so the pair of k-tiles costs 0.5 cyc/row each.
    w8_d = nc.dram_tensor("w8", [KT_ROWS, 2 * 2 * OUTPUT_DIM], F8, kind="ExternalInput")
    out_d = nc.dram_tensor("out", [BATCH, OUTPUT_DIM, S_SHARD], F32, kind="ExternalOutput")
    # DRAM scratch for the theta round-trip (layout change [96,1024]->[3,16384])
    thhi_d = nc.dram_tensor("thhi_s", [96, 1024], BF16, kind="Internal")
    thlo_d = nc.dram_tensor("thlo_s", [96, 1024], BF16, kind="Internal")

    with tile.TileContext(nc) as tc:
        with (
            tc.tile_pool(name="const", bufs=1) as constp,
            tc.tile_pool(name="theta", bufs=1) as thp,
            tc.tile_pool(name="thr", bufs=2) as thrp,
            tc.tile_pool(name="yint", bufs=2) as yp,
            tc.tile_pool(name="tmat", bufs=2) as tp,
            tc.tile_pool(name="outs", bufs=4) as op,
            tc.tile_pool(name="psth", bufs=2, space="PSUM") as pth,
            tc.tile_pool(name="psum", bufs=4, space="PSUM") as pp,
        ):
            w_t = constp.tile([KT_ROWS, NKT * OUTPUT_DIM], F16)
            nc.sync.dma_start(w_t[:], w_d[:])
            pc_t = constp.tile([KT_ROWS, NKT], F32)
            nc.sync.dma_start(pc_t[:], pc_d[:])
            e_t = constp.tile([INPUT_DIM, KT_ROWS], BF16)
            nc.sync.dma_start(e_t[:], e_d[:])
            w8_t = constp.tile([KT_ROWS, 2 * 2 * OUTPUT_DIM], F8)
            nc.sync.dma_start(w8_t[:], w8_d[:])

            for _rep in range(reps):
              # ---- theta stage: flat [96, 1024]; row = 48*b + 16*i + u, u = s-chunk
              xt = thp.tile([96, 1024], F32)
              nc.sync.dma_start(xt[:], x_d[:].rearrange("b i (u c) -> (b i u) c", c=1024))
              sq = thp.tile([96, 1024], F32)
              nc.scalar.activation(sq[:], xt[:], AF.Square)
              r2 = thp.tile([96, 1024], F32)
              nc.scalar.activation(r2[:], sq[:], AF.Sqrt, bias=1.0, scale=-1.0)
              inv = thp.tile([96, 1024], F32)
              nc.vector.reciprocal(inv[:], r2[:])
              q = thp.tile([96, 1024], F32)
              nc.vector.tensor_mul(q[:], xt[:], inv[:])
              asn = thp.tile([96, 1024], F32)
              nc.scalar.activation(asn[:], q[:], AF.Arctan)
              # theta' = (pi/2 - a) * 2^16/(2pi) = 2^14 - a * (2^16/2pi)
              thf = thp.tile([96, 1024], F32)
              nc.scalar.activation(thf[:], asn[:], AF.Copy,
                                   bias=16384.0, scale=float(-TWO16 / (2 * np.pi)))
              # split theta' into bf16 hi+lo (exact reconstruction to 2^-9*32)
              thhi = thp.tile([96, 1024], BF16)
              nc.vector.tensor_copy(thhi[:], thf[:])
              thlo = thp.tile([96, 1024], BF16)
              nc.vector.tensor_tensor(thlo[:], thf[:], thhi[:], ALU.subtract)
              # round-trip via DRAM to land [3, 16384] on partitions 0..2
              nc.sync.dma_start(thhi_d[:], thhi[:])
              nc.sync.dma_start(thlo_d[:], thlo[:])
              thhi_v = thhi_d[:].rearrange("(b i u) c -> b i (u c)", b=2, i=3)
              thlo_v = thlo_d[:].rearrange("(b i u) c -> b i (u c)", b=2, i=3)

              # ---- main loops: groups of 8 chunks share one [3, 8*SC] theta slab
              for g in range(4):
                b = g // 2
                sc0 = (g % 2) * 8
                thr_hi = thrp.tile([INPUT_DIM, 8 * SC], BF16, tag="hi")
                nc.sync.dma_start(thr_hi[:], thhi_v[b, :, sc0 * SC:(sc0 + 8) * SC])
                thr_lo = thrp.tile([INPUT_DIM, 8 * SC], BF16, tag="lo")
                nc.sync.dma_start(thr_lo[:], thlo_v[b, :, sc0 * SC:(sc0 + 8) * SC])
                for u in range(8):
                    sc = sc0 + u
                    # PE broadcast: th3[j,:] = theta'_{j//42}  (fp32 PSUM accum;
                    # matmul output is limited to one PSUM bank = 512 fp32)
                    th3 = pth.tile([KT_ROWS, SC], F32)
                    for hb in range(SC // 512):
                        cs = slice(u * SC + hb * 512, u * SC + hb * 512 + 512)
                        nc.tensor.matmul(th3[:, hb * 512:(hb + 1) * 512],
                                         e_t[:], thr_hi[:, cs],
                                         start=True, stop=False)
                        nc.tensor.matmul(th3[:, hb * 512:(hb + 1) * 512],
                                         e_t[:], thr_lo[:, cs],
                                         start=False, stop=True)
                    # Drain th3 to SBUF once (PSUM reads cost 2x on DVE/ACT,
                    # and Pool cannot read PSUM at all).
                    th3s = thrp.tile([KT_ROWS, SC], F32, tag="th3s")
                    nc.vector.tensor_copy(th3s[:], th3[:])
                    y32 = yp.tile([KT_ROWS, NKT * SC], I32)
                    for kt in range(NKT):
                        # y32 conversions split across Pool (idle) and DVE
                        eng = nc.gpsimd if kt < 4 else nc.vector
                        eng.tensor_scalar(
                            y32[:, kt * SC:(kt + 1) * SC], th3s[:],
                            pc_t[:, kt:kt + 1], 0.25 * TWO16, ALU.mult, ALU.add,
                        )
                    tm = tp.tile([KT_ROWS, NKT * SC], F16)
                    tm8 = tp.tile([KT_ROWS, 2 * SC], F8, tag="tm8")
                    yv = y32[:].bitcast(I16).rearrange("p (n two) -> p n two", two=2)[:, :, 0]
                    sinscale = float(2 * np.pi / TWO16)
                    # fp16 Sin over k-tile column ranges {0,1}, {3}, {5,6};
                    # fp8 Sin for tiles 2 and 4 (DoubleRow operands).
                    for c0, c1 in ((0, 2), (3, 4), (5, 7)):
                        nc.scalar.activation(tm[:, c0 * SC:c1 * SC],
                                             yv[:, c0 * SC:c1 * SC],
                                             AF.Sin, scale=sinscale)
                    for t, kt in enumerate(FP8_KTS):
                        nc.scalar.activation(tm8[:, t * SC:(t + 1) * SC],
                                             yv[:, kt * SC:(kt + 1) * SC],
                                             AF.Sin, scale=sinscale)

                    w8v = w8_t[:].rearrange("p (q t o) -> p q t o", q=2, t=2)
                    tm8v = tm8[:].rearrange("p (t c) -> p t c", t=2)
                    for m in range(2):
                        for half in range(2):
                            ps = pp.tile([128, 512], F32)
                            for ki, kt in enumerate((0, 1, 3, 5, 6)):
                                nc.tensor.matmul(
                                    ps[:],
                                    w_t[:, kt * OUTPUT_DIM + m * 128: kt * OUTPUT_DIM + m * 128 + 128],
                                    tm[:, kt * SC + half * 512: kt * SC + half * 512 + 512],
                                    start=(ki == 0), stop=False,
                                )
                            for q in range(2):
                                nc.tensor.matmul(
                                    ps[:],
                                    w8v[:, q, :, m * 128:m * 128 + 128],
                                    tm8v[:, :, half * 512:half * 512 + 512],
                                    start=False, stop=(q == 1),
                                    perf_mode=mybir.MatmulPerfMode.DoubleRow,
                                )
                            ob = op.tile([128, 512], F32)
                            nc.vector.tensor_copy(ob[:], ps[:])
                            nc.sync.dma_start(
                                out_d[b, m * 128:(m + 1) * 128,
                                      sc * SC + half * 512: sc * SC + half * 512 + 512],
                                ob[:],
                            )
    nc.compile()
    return nc


def _host_prep(coefficients):
    w = (coefficients.astype(np.float64) * WEIGHT_MAGNITUDE).astype(np.float32)
    # w: (256, 3, 257) -> lhsT rows j (i=j//42, p=42*kt+j%42), cols kt*256+o
    wk = np.zeros((KT_ROWS, NKT * OUTPUT_DIM), np.float32)
    j = np.arange(KT_ROWS)
    ii = j // 42
    for kt in range(NKT):
        pp_ = 42 * kt + (j % 42)
        valid = pp_ <= POLY_DEGREE
        # wk[j, kt*256 + o] = w[o, ii[j], pp_[j]]
        wk[valid, kt * OUTPUT_DIM:(kt + 1) * OUTPUT_DIM] = \
            w[:, ii[valid], pp_[valid]].T
    pc = np.zeros((KT_ROWS, NKT), np.float32)
    for kt in range(NKT):
        pc[:, kt] = 42 * kt + (j % 42)
    # ones-selector E for the PE broadcast: e[i, j] = (j // 42 == i)
    import ml_dtypes
    e = (ii[None, :] == np.arange(INPUT_DIM)[:, None]).astype(ml_dtypes.bfloat16)
    # fp8 DoubleRow weights for FP8_KTS with error compensation:
    # w8[:, q*512 + t*256 + o]: q=0 -> e4m3(w), q=1 -> e4m3(w - e4m3(w))
    w8 = np.zeros((KT_ROWS, 2 * 2 * OUTPUT_DIM), ml_dtypes.float8_e4m3)
    for t, kt in enumerate(FP8_KTS):
        wt = wk[:, kt * OUTPUT_DIM:(kt + 1) * OUTPUT_DIM]
        a = wt.astype(ml_dtypes.float8_e4m3)
        b = (wt - a.astype(np.float32)).astype(ml_dtypes.float8_e4m3)
        w8[:, t * OUTPUT_DIM:(t + 1) * OUTPUT_DIM] = a
        w8[:, 512 + t * OUTPUT_DIM:512 + (t + 1) * OUTPUT_DIM] = b
    return wk.astype(np.float16), pc, e, w8


def _get_callable(n_execs=1):
    """Build (once) a jitted shard_map callable running the bass program on 8 cores.

    n_execs>1 compiles a DIFFERENT bass program with the whole device pipeline
    repeated n_execs times (the neuronx_cc_hook only supports one bass_exec
    custom-call per jit, so chaining execs is not possible; on-device reps give
    the same differential-timing semantics with dispatch overhead cancelling).

    Inputs (globals, concat on axis 0 across cores):
      xg [8*2, 3, S_SHARD] f32, wg [8*126, 1792] f16, pcg [8*126, 7] f32,
      eg [8*3, 126] bf16.
    Returns out global [8*2, 256, S_SHARD] f32.
    """
    key = ("fn", n_execs)
    if key in _compiled:
        return _compiled[key]
    import jax
    import jax.numpy as jnp
    from jax.sharding import Mesh, PartitionSpec
    from jax.experimental.shard_map import shard_map
    from concourse import bass2jax
    from concourse.bass2jax import (
        _bass_exec_p, install_neuronx_cc_hook, partition_id_tensor)

    nckey = "nc" if n_execs == 1 else ("nc", n_execs)
    if nckey not in _compiled:
        _compiled[nckey] = _build(reps=n_execs)
    nc = _compiled[nckey]
    install_neuronx_cc_hook()

    pname = nc.partition_id_tensor.name if nc.partition_id_tensor else None
    in_names = ("x", "w", "pc", "e", "w8", "out") + ((pname,) if pname else ())
    out_names = ("out",)
    out_aval = jax.core.ShapedArray((BATCH, OUTPUT_DIM, S_SHARD), np.float32)

    def _body(xs, ws, pcs, es, w8s, zs):
        operands = [xs, ws, pcs, es, w8s, zs]
        if pname:
            operands.append(partition_id_tensor())
        outs = _bass_exec_p.bind(
            *operands,
            out_avals=(out_aval,),
            in_names=in_names,
            out_names=out_names,
            lowering_input_output_aliases=(),
            sim_require_finite=True,
            sim_require_nnan=True,
            nc=nc,
        )
        return outs[0]

    devices = jax.devices()[:N_CORES]
    mesh = Mesh(np.asarray(devices), ("core",))
    fn = jax.jit(shard_map(
        _body, mesh=mesh,
        in_specs=(PartitionSpec("core"),) * 6,
        out_specs=PartitionSpec("core"),
        check_rep=False,
    ))
    _compiled[key] = fn
    return fn


def _make_zeros():
    """Fresh on-device zero output buffers (donated into each kernel call)."""
    import jax
    import jax.numpy as jnp
    from jax.sharding import Mesh, PartitionSpec, NamedSharding

    if "zmk" not in _compiled:
        devices = jax.devices()[:N_CORES]
        mesh = Mesh(np.asarray(devices), ("core",))
        sh = NamedSharding(mesh, PartitionSpec("core"))
        _compiled["zmk"] = jax.jit(
            lambda: jnp.zeros((N_CORES * BATCH, OUTPUT_DIM, S_SHARD), np.float32),
            out_shardings=sh)
    return _compiled["zmk"]()


def _prep_globals(x, coefficients):
    wk, pc, e, w8 = _host_prep(coefficients)
    xg = np.ascontiguousarray(
        np.asarray(x, dtype=np.float32).reshape(BATCH, INPUT_DIM, N_CORES, S_SHARD)
        .transpose(2, 0, 1, 3).reshape(N_CORES * BATCH, INPUT_DIM, S_SHARD))
    wg = np.tile(wk, (N_CORES, 1))
    pcg = np.tile(pc, (N_CORES, 1))
    eg = np.tile(e, (N_CORES, 1))
    w8g = np.tile(w8, (N_CORES, 1))
    return xg, wg, pcg, eg, w8g


def kernel(x, coefficients):
    from concourse import bass2jax

    if "nc" not in _compiled:
        _compiled["nc"] = _build()
    nc = _compiled["nc"]
    wk, pc, e, w8 = _host_prep(coefficients)
    x = np.asarray(x, dtype=np.float32)
    in_maps = [
        {"x": np.ascontiguousarray(x[:, :, c * S_SHARD:(c + 1) * S_SHARD]),
         "w": wk, "pc": pc, "e": e, "w8": w8}
        for c in range(N_CORES)
    ]
    results = bass2jax.run_bass_via_pjrt(nc, in_maps, n_cores=N_CORES)
    out = np.concatenate([results[c]["out"] for c in range(N_CORES)], axis=2)
    return np.ascontiguousarray(out.astype(np.float32))


# revision 31
# speedup vs baseline: 1.6247x; 1.2032x over previous
"""Trainium2 Bass kernel for nn_ChebychevInput.

out[b,o,s] = sum_{i,p} (WEIGHT_MAGNITUDE*coef[o,i,p]) * cos(p*arccos(x[b,i,s]))

Device pipeline per core (s-shard of 16384, both batches):
  theta-stage (tiny, flat [96,1024] layout):
      a = arctan(x/sqrt(1-x^2)) = arcsin(x);  theta = pi/2 - a
      theta' = theta * 2^16/(2pi)   (cycles in 2^16 units)
      split theta' = hi + lo (bf16 pair, exact to ~2^-9 * 32 = 0.0625 units),
      round-trip through DRAM into a [3, 16384] layout (partitions 0..2).
  per (b, s-chunk) of 1024 samples:
      PE    th3[126, SC] (42 rows per i) = E.T @ [theta_hi; theta_lo] --
            two accumulating bf16 matmuls into PSUM (exact ones-selector E)
      DVE   (x7 k-tiles): Y32 = int32(th3_psum * p + 0.25*2^16)   [one pass]
      ACT   one Sin over the int16-bitcast low halfwords: T = sin(2pi*Y/2^16)
            = cos(p*theta)   -> fp16
      PE    out[o,s] accumulated over 7 k-tiles: lhsT = W[126,128] fp16
      Pool  PSUM -> SBUF fp32, DMA -> out
Row packing: k-tile kt row j: i = j//42, p = 42*kt + j%42  (k=126 rows/tile).
"""
import sys

sys.path.insert(0, "/opt/trn_rl_repo")

import numpy as np

BATCH = 2
INPUT_DIM = 3
N_SAMPLES = 131072
OUTPUT_DIM = 256
POLY_DEGREE = 256  # p = 0..256 -> 257 values
N_CORES = 8
S_SHARD = N_SAMPLES // N_CORES  # 16384
SC = 1024                       # sample chunk
NSC = S_SHARD // SC             # 16
NKT = 7                         # k-tiles of 126 rows (3i x 42p)
KT_ROWS = 126
WEIGHT_MAGNITUDE = float(np.sqrt(6.0 / (INPUT_DIM * (POLY_DEGREE + 1))))
TWO16 = 65536.0
FP8_PAIRS = ((2, 4),)  # k-tile pairs computed in fp8 e4m3 DoubleRow
FP8_KTS = tuple(kt for pr in FP8_PAIRS for kt in pr)
F16_KTS = tuple(kt for kt in range(NKT) if kt not in FP8_KTS)
# y32/tm column slot per k-tile: fp16 tiles first (slots 0..len(F16)-1, one
# fp16 Sin over a contiguous range), fp8 tiles after (one fp8 Sin).
SLOT = {kt: s for s, kt in enumerate(F16_KTS + FP8_KTS)}

_compiled = {}


def _build(reps=1):
    import concourse.tile as tile
    from concourse import bacc, mybir

    F32 = mybir.dt.float32
    F16 = mybir.dt.float16
    BF16 = mybir.dt.bfloat16
    I32 = mybir.dt.int32
    I16 = mybir.dt.int16
    AF = mybir.ActivationFunctionType
    ALU = mybir.AluOpType

    F8 = mybir.dt.float8e4
    nc = bacc.Bacc("TRN2", target_bir_lowering=False, debug=False)
    x_d = nc.dram_tensor("x", [BATCH, INPUT_DIM, S_SHARD], F32, kind="ExternalInput")
    w_d = nc.dram_tensor("w", [KT_ROWS, NKT * OUTPUT_DIM], F16, kind="ExternalInput")
    pc_d = nc.dram_tensor("pc", [KT_ROWS, NKT], F32, kind="ExternalInput")
    e_d = nc.dram_tensor("e", [INPUT_DIM, KT_ROWS], BF16, kind="ExternalInput")
    # fp8 weights for k-tile pairs: cols (P, q, t, o); q=0 -> e4m3(w),
    # q=1 -> e4m3(w - e4m3(w)) error compensation (DoubleRow operands).
    NP8 = len(FP8_PAIRS)
    w8_d = nc.dram_tensor("w8", [KT_ROWS, NP8 * 2 * 2 * OUTPUT_DIM], F8,
                          kind="ExternalInput")
    out_d = nc.dram_tensor("out", [BATCH, OUTPUT_DIM, S_SHARD], F32, kind="ExternalOutput")
    # DRAM scratch for the theta round-trip; double-buffered by rep parity so
    # rep r+1's theta stage does not WAR-stall on rep r's tail reads.
    thhi_ds = [nc.dram_tensor(f"thhi_s{i}", [96, 1024], BF16, kind="Internal")
               for i in range(2)]
    thlo_ds = [nc.dram_tensor(f"thlo_s{i}", [96, 1024], BF16, kind="Internal")
               for i in range(2)]

    with tile.TileContext(nc) as tc:
        with (
            tc.tile_pool(name="const", bufs=1) as constp,
            tc.tile_pool(name="theta", bufs=1) as thp,
            tc.tile_pool(name="thr", bufs=2) as thrp,
            tc.tile_pool(name="yint", bufs=2) as yp,
            tc.tile_pool(name="tmat", bufs=3) as tp,
            tc.tile_pool(name="outs", bufs=4) as op,
            tc.tile_pool(name="psth", bufs=2, space="PSUM") as pth,
            tc.tile_pool(name="psum", bufs=4, space="PSUM") as pp,
        ):
            w_t = constp.tile([KT_ROWS, NKT * OUTPUT_DIM], F16)
            nc.sync.dma_start(w_t[:], w_d[:])
            pc_t = constp.tile([KT_ROWS, NKT], F32)
            nc.sync.dma_start(pc_t[:], pc_d[:])
            e_t = constp.tile([INPUT_DIM, KT_ROWS], BF16)
            nc.sync.dma_start(e_t[:], e_d[:])
            w8_t = constp.tile([KT_ROWS, NP8 * 2 * 2 * OUTPUT_DIM], F8)
            nc.sync.dma_start(w8_t[:], w8_d[:])

            for _rep in range(reps):
              # ---- theta stage: flat [96, 1024]; row = 48*b + 16*i + u, u = s-chunk
              xt = thp.tile([96, 1024], F32)
              nc.sync.dma_start(xt[:], x_d[:].rearrange("b i (u c) -> (b i u) c", c=1024))
              sq = thp.tile([96, 1024], F32)
              nc.scalar.activation(sq[:], xt[:], AF.Square)
              r2 = thp.tile([96, 1024], F32)
              nc.scalar.activation(r2[:], sq[:], AF.Sqrt, bias=1.0, scale=-1.0)
              inv = thp.tile([96, 1024], F32)
              nc.vector.reciprocal(inv[:], r2[:])
              q = thp.tile([96, 1024], F32)
              nc.vector.tensor_mul(q[:], xt[:], inv[:])
              asn = thp.tile([96, 1024], F32)
              nc.scalar.activation(asn[:], q[:], AF.Arctan)
              # theta' = (pi/2 - a) * 2^16/(2pi) = 2^14 - a * (2^16/2pi)
              thf = thp.tile([96, 1024], F32)
              nc.scalar.activation(thf[:], asn[:], AF.Copy,
                                   bias=16384.0, scale=float(-TWO16 / (2 * np.pi)))
              # split theta' into bf16 hi+lo (exact reconstruction to 2^-9*32)
              thhi = thp.tile([96, 1024], BF16)
              nc.vector.tensor_copy(thhi[:], thf[:])
              thlo = thp.tile([96, 1024], BF16)
              nc.vector.tensor_tensor(thlo[:], thf[:], thhi[:], ALU.subtract)
              # round-trip via DRAM to land [3, 16384] on partitions 0..2
              thhi_d = thhi_ds[_rep % 2]
              thlo_d = thlo_ds[_rep % 2]
              nc.sync.dma_start(thhi_d[:], thhi[:])
              nc.sync.dma_start(thlo_d[:], thlo[:])
              thhi_v = thhi_d[:].rearrange("(b i u) c -> b i (u c)", b=2, i=3)
              thlo_v = thlo_d[:].rearrange("(b i u) c -> b i (u c)", b=2, i=3)

              # ---- main loops: groups of 4 chunks share one [3, 4*SC] theta slab
              for g in range(8):
                b = g // 4
                sc0 = (g % 4) * 4
                thr_hi = thrp.tile([INPUT_DIM, 4 * SC], BF16, tag="hi")
                nc.sync.dma_start(thr_hi[:], thhi_v[b, :, sc0 * SC:(sc0 + 4) * SC])
                thr_lo = thrp.tile([INPUT_DIM, 4 * SC], BF16, tag="lo")
                nc.sync.dma_start(thr_lo[:], thlo_v[b, :, sc0 * SC:(sc0 + 4) * SC])
                for u in range(4):
                    sc = sc0 + u
                    # PE broadcast: th3[j,:] = theta'_{j//42}  (fp32 PSUM accum;
                    # matmul output is limited to one PSUM bank = 512 fp32)
                    th3 = pth.tile([KT_ROWS, SC], F32)
                    for hb in range(SC // 512):
                        cs = slice(u * SC + hb * 512, u * SC + hb * 512 + 512)
                        nc.tensor.matmul(th3[:, hb * 512:(hb + 1) * 512],
                                         e_t[:], thr_hi[:, cs],
                                         start=True, stop=False)
                        nc.tensor.matmul(th3[:, hb * 512:(hb + 1) * 512],
                                         e_t[:], thr_lo[:, cs],
                                         start=False, stop=True)
                    # Drain th3 to SBUF once (PSUM reads cost 2x on DVE/ACT,
                    # and Pool cannot read PSUM at all).
                    th3s = thrp.tile([KT_ROWS, SC], F32, tag="th3s")
                    nc.vector.tensor_copy(th3s[:], th3[:])
                    y32 = yp.tile([KT_ROWS, NKT * SC], I32)
                    for kt in range(NKT):
                        # y32 conversions split across Pool (idle) and DVE;
                        # written at the k-tile's column SLOT
                        eng = nc.gpsimd if kt < 4 else nc.vector
                        s = SLOT[kt]
                        eng.tensor_scalar(
                            y32[:, s * SC:(s + 1) * SC], th3s[:],
                            pc_t[:, kt:kt + 1], 0.25 * TWO16, ALU.mult, ALU.add,
                        )
                    NF16 = len(F16_KTS)
                    tm = tp.tile([KT_ROWS, NF16 * SC], F16)
                    tm8 = tp.tile([KT_ROWS, 2 * NP8 * SC], F8, tag="tm8")
                    yv = y32[:].bitcast(I16).rearrange("p (n two) -> p n two", two=2)[:, :, 0]
                    sinscale = float(2 * np.pi / TWO16)
                    # slots 0..NF16-1 are the fp16 tiles (one Sin); the fp8
                    # DoubleRow tiles follow (one Sin).
                    nc.scalar.activation(tm[:], yv[:, 0:NF16 * SC],
                                         AF.Sin, scale=sinscale)
                    nc.scalar.activation(tm8[:], yv[:, NF16 * SC:NKT * SC],
                                         AF.Sin, scale=sinscale)

                    w8v = w8_t[:].rearrange("p (P q t o) -> p P q t o", P=NP8, q=2, t=2)
                    for m in range(2):
                        for half in range(2):
                            ps = pp.tile([128, 512], F32)
                            for ki in range(NF16):
                                kt = F16_KTS[ki]
                                nc.tensor.matmul(
                                    ps[:],
                                    w_t[:, kt * OUTPUT_DIM + m * 128: kt * OUTPUT_DIM + m * 128 + 128],
                                    tm[:, ki * SC + half * 512: ki * SC + half * 512 + 512],
                                    start=(ki == 0), stop=False,
                                )
                            for P in range(NP8):
                                tm8v = tm8[:, 2 * P * SC:2 * (P + 1) * SC] \
                                    .rearrange("p (t c) -> p t c", t=2)
                                for q in range(2):
                                    nc.tensor.matmul(
                                        ps[:],
                                        w8v[:, P, q, :, m * 128:m * 128 + 128],
                                        tm8v[:, :, half * 512:half * 512 + 512],
                                        start=False,
                                        stop=(P == NP8 - 1 and q == 1),
                                        perf_mode=mybir.MatmulPerfMode.DoubleRow,
                                    )
                            ob = op.tile([128, 512], F32)
                            nc.vector.tensor_copy(ob[:], ps[:])
                            nc.sync.dma_start(
                                out_d[b, m * 128:(m + 1) * 128,
                                      sc * SC + half * 512: sc * SC + half * 512 + 512],
                                ob[:],
                            )
    nc.compile()
    return nc


def _host_prep(coefficients):
    w = (coefficients.astype(np.float64) * WEIGHT_MAGNITUDE).astype(np.float32)
    # w: (256, 3, 257) -> lhsT rows j (i=j//42, p=42*kt+j%42), cols kt*256+o
    wk = np.zeros((KT_ROWS, NKT * OUTPUT_DIM), np.float32)
    j = np.arange(KT_ROWS)
    ii = j // 42
    for kt in range(NKT):
        pp_ = 42 * kt + (j % 42)
        valid = pp_ <= POLY_DEGREE
        # wk[j, kt*256 + o] = w[o, ii[j], pp_[j]]
        wk[valid, kt * OUTPUT_DIM:(kt + 1) * OUTPUT_DIM] = \
            w[:, ii[valid], pp_[valid]].T
    pc = np.zeros((KT_ROWS, NKT), np.float32)
    for kt in range(NKT):
        pc[:, kt] = 42 * kt + (j % 42)
    # ones-selector E for the PE broadcast: e[i, j] = (j // 42 == i)
    import ml_dtypes
    e = (ii[None, :] == np.arange(INPUT_DIM)[:, None]).astype(ml_dtypes.bfloat16)
    # fp8 DoubleRow weights for FP8_PAIRS with error compensation:
    # w8[:, P*1024 + q*512 + t*256 + o]: q=0 -> e4m3(w), q=1 -> e4m3(w - e4m3(w))
    w8 = np.zeros((KT_ROWS, len(FP8_PAIRS) * 2 * 2 * OUTPUT_DIM),
                  ml_dtypes.float8_e4m3)
    for P, pair in enumerate(FP8_PAIRS):
        for t, kt in enumerate(pair):
            wt = wk[:, kt * OUTPUT_DIM:(kt + 1) * OUTPUT_DIM]
            a = wt.astype(ml_dtypes.float8_e4m3)
            b = (wt - a.astype(np.float32)).astype(ml_dtypes.float8_e4m3)
            base = P * 1024
            w8[:, base + t * OUTPUT_DIM:base + (t + 1) * OUTPUT_DIM] = a
            w8[:, base + 512 + t * OUTPUT_DIM:base + 512 + (t + 1) * OUTPUT_DIM] = b
    return wk.astype(np.float16), pc, e, w8


def _get_callable(n_execs=1):
    """Build (once) a jitted shard_map callable running the bass program on 8 cores.

    n_execs>1 compiles a DIFFERENT bass program with the whole device pipeline
    repeated n_execs times (the neuronx_cc_hook only supports one bass_exec
    custom-call per jit, so chaining execs is not possible; on-device reps give
    the same differential-timing semantics with dispatch overhead cancelling).

    Inputs (globals, concat on axis 0 across cores):
      xg [8*2, 3, S_SHARD] f32, wg [8*126, 1792] f16, pcg [8*126, 7] f32,
      eg [8*3, 126] bf16.
    Returns out global [8*2, 256, S_SHARD] f32.
    """
    key = ("fn", n_execs)
    if key in _compiled:
        return _compiled[key]
    import jax
    import jax.numpy as jnp
    from jax.sharding import Mesh, PartitionSpec
    from jax.experimental.shard_map import shard_map
    from concourse import bass2jax
    from concourse.bass2jax import (
        _bass_exec_p, install_neuronx_cc_hook, partition_id_tensor)

    nckey = "nc" if n_execs == 1 else ("nc", n_execs)
    if nckey not in _compiled:
        _compiled[nckey] = _build(reps=n_execs)
    nc = _compiled[nckey]
    install_neuronx_cc_hook()

    pname = nc.partition_id_tensor.name if nc.partition_id_tensor else None
    in_names = ("x", "w", "pc", "e", "w8", "out") + ((pname,) if pname else ())
    out_names = ("out",)
    out_aval = jax.core.ShapedArray((BATCH, OUTPUT_DIM, S_SHARD), np.float32)

    def _body(xs, ws, pcs, es, w8s, zs):
        operands = [xs, ws, pcs, es, w8s, zs]
        if pname:
            operands.append(partition_id_tensor())
        outs = _bass_exec_p.bind(
            *operands,
            out_avals=(out_aval,),
            in_names=in_names,
            out_names=out_names,
            lowering_input_output_aliases=(),
            sim_require_finite=True,
            sim_require_nnan=True,
            nc=nc,
        )
        return outs[0]

    devices = jax.devices()[:N_CORES]
    mesh = Mesh(np.asarray(devices), ("core",))
    fn = jax.jit(shard_map(
        _body, mesh=mesh,
        in_specs=(PartitionSpec("core"),) * 6,
        out_specs=PartitionSpec("core"),
        check_rep=False,
    ))
    _compiled[key] = fn
    return fn


def _make_zeros():
    """Fresh on-device zero output buffers (donated into each kernel call)."""
    import jax
    import jax.numpy as jnp
    from jax.sharding import Mesh, PartitionSpec, NamedSharding

    if "zmk" not in _compiled:
        devices = jax.devices()[:N_CORES]
        mesh = Mesh(np.asarray(devices), ("core",))
        sh = NamedSharding(mesh, PartitionSpec("core"))
        _compiled["zmk"] = jax.jit(
            lambda: jnp.zeros((N_CORES * BATCH, OUTPUT_DIM, S_SHARD), np.float32),
            out_shardings=sh)
    return _compiled["zmk"]()


def _prep_globals(x, coefficients):
    wk, pc, e, w8 = _host_prep(coefficients)
    xg = np.ascontiguousarray(
        np.asarray(x, dtype=np.float32).reshape(BATCH, INPUT_DIM, N_CORES, S_SHARD)
        .transpose(2, 0, 1, 3).reshape(N_CORES * BATCH, INPUT_DIM, S_SHARD))
    wg = np.tile(wk, (N_CORES, 1))
    pcg = np.tile(pc, (N_CORES, 1))
    eg = np.tile(e, (N_CORES, 1))
    w8g = np.tile(w8, (N_CORES, 1))
    return xg, wg, pcg, eg, w8g


def kernel(x, coefficients):
    from concourse import bass2jax

    if "nc" not in _compiled:
        _compiled["nc"] = _build()
    nc = _compiled["nc"]
    wk, pc, e, w8 = _host_prep(coefficients)
    x = np.asarray(x, dtype=np.float32)
    in_maps = [
        {"x": np.ascontiguousarray(x[:, :, c * S_SHARD:(c + 1) * S_SHARD]),
         "w": wk, "pc": pc, "e": e, "w8": w8}
        for c in range(N_CORES)
    ]
    results = bass2jax.run_bass_via_pjrt(nc, in_maps, n_cores=N_CORES)
    out = np.concatenate([results[c]["out"] for c in range(N_CORES)], axis=2)
    return np.ascontiguousarray(out.astype(np.float32))
